# revision 21
# baseline (speedup 1.0000x reference)
"""Multi-head latent attention (MLA) Bass kernel for 8 TRN2 NeuronCores.

Sharding: tensor-parallel over heads x data-parallel over batch.
Core c (0..7) owns batch b = c//4 and head group g = c%4 (8 heads of 32).
Each core computes, for its batch:
    latentT = (hs @ Wc)^T          (seq-sharded + AllGather within batch group)
    qT_h, kT_h (RoPE'd, transposed [head_dim, seq]) and v for its 8 heads
    attention with transposed scores [s_k, s_q]; the softmax normalizer is
    accumulated on VectorE (bf16 chunk adds) + ONE ones-matmul per tile
    o-proj fused into the attention loop: partial out -> [S, D] bf16
Host sums the 4 partials per batch in fp32.

Query 512-blocks are PERMUTED per core so the core's latent shard equals
query blocks 0-1: the latent projection reuses the q-proj activation tiles
(no separate hsl load). cos/sin tables are permuted to match; the host
un-permutes output rows.

DMA: all streamed tensors are packed partition-major on the host so each
DMA is one long contiguous run per partition (4-32KB packets, not 512B).
Queues: hs stream + out writes on qSync; weights on qScalar; latent
AllGather staging on the gpsimd SWDGE queue.
"""

import sys

for _p in ("/opt/trn_rl_repo", "/root/.axon_site/_ro/trn_rl_repo"):
    if _p not in sys.path:
        sys.path.insert(0, _p)

import numpy as np
import ml_dtypes

import concourse.bacc as bacc
import concourse.mybir as mybir
import concourse.tile as tile
from concourse import bass_isa
from concourse.bass_utils import run_bass_kernel_spmd

BF = mybir.dt.bfloat16
F32 = mybir.dt.float32
BF_NP = ml_dtypes.bfloat16

# Full-problem constants (hardcoded per the self-contained-kernel contract).
D_MODEL = 4096
D_LATENT = 512
NUM_HEADS = 32
HEAD_DIM = 128
ROPE_THETA = 10000.0
BATCH, SEQ = 2, 2048
N_CORES = 8
HEADS_PER_CORE = NUM_HEADS // 4  # 4 head groups x 2 batches = 8 cores
NA = 256                         # q-proj seq tile width
Q4 = 4                           # d_model quarters for hs/wc streaming


def build_nc(S=SEQ, D=D_MODEL, L=D_LATENT, H=HEADS_PER_CORE, Dh=HEAD_DIM,
             NC=512):
    """Build the single-core Bass program (SPMD across 8 cores)."""
    KD = D // 128     # contraction chunks over d_model
    KQ = KD // Q4     # chunks per quarter (8)
    LD = L // 128     # contraction chunks over d_latent
    JA = S // NA      # seq chunks in projection phase
    JC = S // NC      # seq chunks in attention phase
    SK = S // 128     # key-position chunks
    HD1 = H * Dh      # this core's total head width (1024)
    ND = D // NC      # output-column chunks
    SQ = S // 4       # this core's latent shard width (batch group of 4)
    NB = 512          # seq chunk width in phase B
    TPC = NC // 128   # seq tiles per attention chunk
    half = Dh // 2
    assert SK % 2 == 0 and NC == 512 and SQ == 2 * NA

    nc = bacc.Bacc("TRN2", target_bir_lowering=False)

    # partition-major packed streams (see host_inputs)
    hsq_d = nc.declare_dram_parameter("hsQ", [JA, Q4, 128, KQ * NA], BF,
                                      isOutput=False)
    wq_d = nc.declare_dram_parameter("WqP", [H, 128, KD * Dh], BF, isOutput=False)
    wc_d = nc.declare_dram_parameter("WcP", [Q4, 128, KQ * L], BF, isOutput=False)
    wk_d = nc.declare_dram_parameter("Wk", [L, HD1], BF, isOutput=False)
    wv_d = nc.declare_dram_parameter("Wv", [L, HD1], BF, isOutput=False)
    wo_d = nc.declare_dram_parameter("Wo", [HD1, D], BF, isOutput=False)
    cosq_d = nc.declare_dram_parameter("cosq", [Dh, S], BF, isOutput=False)
    sinq_d = nc.declare_dram_parameter("sinq", [Dh, S], BF, isOutput=False)
    cosk_d = nc.declare_dram_parameter("cosk", [Dh, S], BF, isOutput=False)
    sink_d = nc.declare_dram_parameter("sink", [Dh, S], BF, isOutput=False)
    out_d = nc.declare_dram_parameter("out", [S, D], BF, isOutput=True)
    latq_d = nc.dram_tensor("latq_dram", [L, SQ], BF)
    latg_d = nc.dram_tensor("latg_dram", [4 * L, SQ], BF)

    Exp = mybir.ActivationFunctionType.Exp

    with tile.TileContext(nc) as tc:
        with tc.tile_pool(name="consts", bufs=1) as const_pool, \
             tc.tile_pool(name="qT", bufs=1) as qT_pool:
            ones_sk = const_pool.tile([128, 128], BF)
            nc.vector.memset(ones_sk[:], 1.0)
            # preload the Exp activation table so the first real exp
            # (phase C) doesn't pay the ~2.7us table-load latency
            expwarm = const_pool.tile([128, 8], BF)
            nc.scalar.activation(expwarm[:], ones_sk[:, 0:8],
                                 mybir.ActivationFunctionType.Exp)
            qT_t = [qT_pool.tile([Dh, S], BF, name=f"qT{h}") for h in range(H)]

            # ========== Phase A: latent shard + AllGather; qT (RoPE) ==========
            with tc.tile_pool(name="wqA", bufs=1) as wqA_pool, \
                 tc.tile_pool(name="wcA", bufs=1) as wcA_pool, \
                 tc.tile_pool(name="hsq", bufs=3 * Q4) as hsq_pool, \
                 tc.tile_pool(name="ropeq", bufs=1) as ropeq_pool, \
                 tc.tile_pool(name="qraw", bufs=4) as qraw_pool, \
                 tc.tile_pool(name="tmpA", bufs=4) as tmpA_pool, \
                 tc.tile_pool(name="lq", bufs=4) as lq_pool, \
                 tc.tile_pool(name="psA", bufs=8, space="PSUM") as psA_pool:

                hs_tiles = {}

                # PE warm-up: dummy matmuls while the first DMAs stream in;
                # keeps the HAM clock-gate warm so real matmuls start at
                # full rate instead of 1.2 GHz
                warm_ps = psA_pool.tile([128, 128], F32, tag="psA",
                                        name="warmps")
                for _w in range(40):
                    nc.tensor.matmul(warm_ps[:], ones_sk[:], ones_sk[:],
                                     start=(_w == 0), stop=(_w == 39))

                def load_hs(j):
                    qs = []
                    for qd in range(Q4):
                        t = hsq_pool.tile([128, KQ * NA], BF, tag="hsq",
                                          name=f"hs_{j}_{qd}")
                        nc.sync.dma_start(out=t[:], in_=hsq_d[j, qd])
                        qs.append(t)
                    hs_tiles[j] = qs

                def hs_sl(j, kd):
                    return hs_tiles[j][kd // KQ][:, (kd % KQ) * NA:
                                                 (kd % KQ + 1) * NA]

                # startup: hs(j0,j1) quarters on the sync queue in latent
                # consumption order; wc on the scalar queue in parallel
                wc_t = [wcA_pool.tile([128, KQ * L], BF, name=f"wc{qd}")
                        for qd in range(Q4)]
                for j in (0, 1):
                    hs_tiles[j] = [hsq_pool.tile([128, KQ * NA], BF, tag="hsq",
                                                 name=f"hs_{j}_{qd}")
                                   for qd in range(Q4)]
                for qd in range(Q4):
                    for j in (0, 1):
                        nc.sync.dma_start(out=hs_tiles[j][qd][:],
                                          in_=hsq_d[j, qd])
                for qd in range(Q4):
                    nc.scalar.dma_start(out=wc_t[qd][:], in_=wc_d[qd])

                def wc_sl(kd, ld):
                    return wc_t[kd // KQ][:, (kd % KQ) * L + ld * 128:
                                          (kd % KQ) * L + (ld + 1) * 128]

                # --- latent shard = query blocks 0-1 (per-core permuted) ---
                # quarter-major accumulation: consume each d_model quarter
                # fully before needing the next one's DMA
                ps_l = {}
                for jq in range(2):
                    for ld in range(LD):
                        ps_l[(jq, ld)] = psA_pool.tile(
                            [128, NA], F32, tag="psA", name=f"psLq{jq}_{ld}")
                for qd in range(Q4):
                    for jq in range(2):
                        for ld in range(LD):
                            for i in range(KQ):
                                kd = qd * KQ + i
                                nc.tensor.matmul(
                                    ps_l[(jq, ld)][:], wc_sl(kd, ld),
                                    hs_sl(jq, kd),
                                    start=(kd == 0), stop=(kd == KD - 1))
                for jq in range(2):
                    qq = slice(jq * NA, (jq + 1) * NA)
                    for ld in range(LD):
                        lq = lq_pool.tile([128, NA], BF, tag="lq",
                                          name=f"lq{jq}_{ld}")
                        nc.scalar.copy(lq[:], ps_l[(jq, ld)][:])
                        nc.gpsimd.dma_start(
                            out=latq_d[ld * 128:(ld + 1) * 128, qq],
                            in_=lq[:])
                nc.gpsimd.collective_compute(
                    "AllGather",
                    mybir.AluOpType.bypass,
                    replica_groups=[[0, 1, 2, 3], [4, 5, 6, 7]],
                    ins=[latq_d[:]],
                    outs=[latg_d[:]],
                )

                # --- qT loop; per-head wq so h=0 starts early ---
                cosq_sb = ropeq_pool.tile([Dh, S], BF)
                sinq_sb = ropeq_pool.tile([Dh, S], BF)
                wq_t = [wqA_pool.tile([128, KD * Dh], BF, name=f"wq{h}")
                        for h in range(H)]
                nc.scalar.dma_start(out=wq_t[0][:], in_=wq_d[0])
                nc.scalar.dma_start(out=cosq_sb[:], in_=cosq_d[:])
                nc.scalar.dma_start(out=sinq_sb[:], in_=sinq_d[:])
                for h in range(1, H):
                    nc.scalar.dma_start(out=wq_t[h][:], in_=wq_d[h])

                for j in range(JA):
                    if 2 <= j + 2 < JA:
                        load_hs(j + 2)
                    jj = slice(j * NA, (j + 1) * NA)
                    for h in range(H):
                        ps = psA_pool.tile([128, NA], F32, tag="psA",
                                           name=f"psQ{j}_{h}")
                        for kd in range(KD):
                            nc.tensor.matmul(
                                ps[:],
                                wq_t[h][:, kd * Dh:(kd + 1) * Dh],
                                hs_sl(j, kd),
                                start=(kd == 0), stop=(kd == KD - 1))
                        qr = qraw_pool.tile([128, NA], BF, tag="qraw",
                                            name=f"qr{j}_{h}")
                        nc.scalar.copy(qr[:], ps[:])
                        t2 = tmpA_pool.tile([128, NA], BF, tag="t2",
                                            name=f"t2q{j}_{h}")
                        t1 = tmpA_pool.tile([128, NA], BF, tag="t1",
                                            name=f"t1q{j}_{h}")
                        nc.vector.tensor_mul(t1[:], qr[:], cosq_sb[:, jj])
                        nc.vector.tensor_mul(t2[0:half, :], ps[half:Dh, :],
                                             sinq_sb[0:half, jj])
                        nc.vector.tensor_mul(t2[half:Dh, :], ps[0:half, :],
                                             sinq_sb[half:Dh, jj])
                        nc.vector.tensor_add(qT_t[h][:, jj], t1[:], t2[:])

            # ========== Phase B: kT (RoPE) + v;  Phase C+D fused ==========
            with tc.tile_pool(name="kT", bufs=1) as kT_pool, \
                 tc.tile_pool(name="v", bufs=1) as v_pool:
                kT_t = [kT_pool.tile([Dh, S], BF, name=f"kT{h}")
                        for h in range(H)]
                v_t = [v_pool.tile([128, HD1], BF, name=f"v{i}")
                       for i in range(SK)]

                with tc.tile_pool(name="latentT", bufs=1) as latent_pool, \
                     tc.tile_pool(name="wkv", bufs=1) as wkv_pool, \
                     tc.tile_pool(name="ropek", bufs=1) as ropek_pool, \
                     tc.tile_pool(name="kraw", bufs=4) as kraw_pool, \
                     tc.tile_pool(name="tmpB", bufs=4) as tmpB_pool, \
                     tc.tile_pool(name="psB", bufs=4, space="PSUM") as psB_pool:

                    latentT_t = [latent_pool.tile([128, S], BF, name=f"latT{ld}")
                                 for ld in range(LD)]
                    for ld in range(LD):
                        for r in range(4):
                            nc.gpsimd.dma_start(
                                out=latentT_t[ld][:, r * SQ:(r + 1) * SQ],
                                in_=latg_d[r * L + ld * 128:
                                           r * L + (ld + 1) * 128, :])
                    wk_t = [wkv_pool.tile([128, HD1], BF, name=f"wk{ld}")
                            for ld in range(LD)]
                    wv_t = [wkv_pool.tile([128, HD1], BF, name=f"wv{ld}")
                            for ld in range(LD)]
                    for ld in range(LD):
                        nc.sync.dma_start(
                            out=wk_t[ld][:], in_=wk_d[ld * 128:(ld + 1) * 128, :])
                        nc.sync.dma_start(
                            out=wv_t[ld][:], in_=wv_d[ld * 128:(ld + 1) * 128, :])
                    cosk_sb = ropek_pool.tile([Dh, S], BF)
                    sink_sb = ropek_pool.tile([Dh, S], BF)
                    nc.sync.dma_start(out=cosk_sb[:], in_=cosk_d[:])
                    nc.sync.dma_start(out=sink_sb[:], in_=sink_d[:])

                    # kT (h=0 first so attention starts early); v-expansion
                    # groups interleaved per head so the PE covers the DVE
                    # RoPE latency instead of stalling on psB reuse
                    NV = 512
                    for h in range(H):
                        for j in range(S // NB):
                            jj = slice(j * NB, (j + 1) * NB)
                            ps = psB_pool.tile([128, NB], F32, tag="psB",
                                               name=f"psK{h}_{j}")
                            for ld in range(LD):
                                nc.tensor.matmul(
                                    ps[:], wk_t[ld][:, h * Dh:(h + 1) * Dh],
                                    latentT_t[ld][:, jj],
                                    start=(ld == 0), stop=(ld == LD - 1))
                            kr = kraw_pool.tile([128, NB], BF, tag="kraw",
                                                name=f"kr{h}_{j}")
                            nc.scalar.copy(kr[:], ps[:])
                            t2 = tmpB_pool.tile([128, NB], BF, tag="t2b",
                                                name=f"t2k{h}_{j}")
                            t1 = tmpB_pool.tile([128, NB], BF, tag="t1b",
                                                name=f"t1k{h}_{j}")
                            nc.vector.tensor_mul(t1[:], kr[:], cosk_sb[:, jj])
                            nc.vector.tensor_mul(t2[0:half, :], ps[half:Dh, :],
                                                 sink_sb[0:half, jj])
                            nc.vector.tensor_mul(t2[half:Dh, :], ps[0:half, :],
                                                 sink_sb[half:Dh, jj])
                            nc.vector.tensor_add(kT_t[h][:, jj], t1[:], t2[:])
                        for k in range(4):
                            i = 2 * h + k // 2
                            cch = k % 2
                            cc = slice(cch * NV, (cch + 1) * NV)
                            ps = psB_pool.tile([128, NV], F32, tag="psB",
                                               name=f"psV{i}_{cch}")
                            for ld in range(LD):
                                nc.tensor.matmul(
                                    ps[:],
                                    latentT_t[ld][:, i * 128:(i + 1) * 128],
                                    wv_t[ld][:, cc],
                                    start=(ld == 0), stop=(ld == LD - 1))
                            nc.scalar.copy(v_t[i][:, cc], ps[:])

                # ===== Phase C+D fused: attention + output projection =====
                with tc.tile_pool(name="wo", bufs=1) as wo_pool, \
                     tc.tile_pool(name="ats", bufs=2 * H) as ats_pool, \
                     tc.tile_pool(name="ET", bufs=8) as et_pool, \
                     tc.tile_pool(name="accp", bufs=1) as acc_pool, \
                     tc.tile_pool(name="rinv", bufs=1) as rinv_pool, \
                     tc.tile_pool(name="outst", bufs=3) as outst_pool, \
                     tc.tile_pool(name="pssc", bufs=2, space="PSUM") as pssc_pool, \
                     tc.tile_pool(name="pspv", bufs=1, space="PSUM") as pspv_pool, \
                     tc.tile_pool(name="psr", bufs=1, space="PSUM") as psr_pool, \
                     tc.tile_pool(name="psD", bufs=2, space="PSUM") as psD_pool:

                    wo_t = [wo_pool.tile([128, D], BF, name=f"wo{h}")
                            for h in range(H)]
                    for h in range(H):
                        nc.sync.dma_start(out=wo_t[h][:],
                                          in_=wo_d[h * 128:(h + 1) * 128, :])

                    ats_t = {}   # (h, jc) -> [Dh, NC] bf16 tile
                    st_box = {}  # (jcp, tl) -> [128, 2*NC] staging tile

                    def emit_oproj_group(jcp, g):
                        """One o-proj PSUM group: local seq tile g//ND, out
                        cols g%ND, contracting all H heads. Output staged in
                        ncol pairs for 2KB-per-partition DMA runs."""
                        tl = g // ND
                        ncol = g % ND
                        t_abs = jcp * TPC + tl
                        tt = slice(t_abs * 128, (t_abs + 1) * 128)
                        ps = psD_pool.tile([128, NC], F32, tag="psD",
                                           name=f"psD{jcp}_{g}")
                        for h in range(H):
                            nc.tensor.matmul(
                                ps[:],
                                ats_t[(h, jcp)][:, tl * 128:(tl + 1) * 128],
                                wo_t[h][:, ncol * NC:(ncol + 1) * NC],
                                start=(h == 0), stop=(h == H - 1))
                        if ncol % 2 == 0:
                            st = outst_pool.tile([128, 2 * NC], BF, tag="outst",
                                                 name=f"outst{jcp}_{g}")
                            st_box[(jcp, tl)] = st
                            nc.vector.tensor_copy(st[:, 0:NC], ps[:])
                        else:
                            st = st_box[(jcp, tl)]
                            nc.vector.tensor_copy(st[:, NC:2 * NC], ps[:])
                            nc.sync.dma_start(
                                out=out_d[tt, (ncol - 1) * NC:(ncol + 1) * NC],
                                in_=st[:])

                    GPH = (TPC * ND) // H  # o-proj groups per h slot
                    for jc in range(JC):
                        jj = slice(jc * NC, (jc + 1) * NC)
                        for h in range(H):
                            # scores (transposed: keys on partitions) + exp;
                            # pv matmuls trail one chunk-pair behind so the
                            # PE stream never outruns ScalarE's exp rate
                            pv = pspv_pool.tile([Dh, NC], F32, tag="pv",
                                                name=f"pv{h}_{jc}")
                            ets = []

                            def pv_pair(i2p):
                                for i in (2 * i2p, 2 * i2p + 1):
                                    sl = ets[i // 2][:, (i % 2) * NC:
                                                     (i % 2 + 1) * NC]
                                    nc.tensor.matmul(
                                        pv[:], v_t[i][:, h * Dh:(h + 1) * Dh],
                                        sl, start=(i == 0),
                                        stop=(i == SK - 1))

                            for i2 in range(SK // 2):
                                ps2 = pssc_pool.tile([128, 2 * NC], F32,
                                                     tag="sc",
                                                     name=f"sc{h}_{jc}_{i2}")
                                for p in range(2):
                                    i = i2 * 2 + p
                                    nc.tensor.matmul(
                                        ps2[:, p * NC:(p + 1) * NC],
                                        kT_t[h][:, i * 128:(i + 1) * 128],
                                        qT_t[h][:, jj],
                                        start=True, stop=True)
                                et = et_pool.tile([128, 2 * NC], BF, tag="ET",
                                                  name=f"et{h}_{jc}_{i2}")
                                nc.scalar.activation(et[:], ps2[:], Exp)
                                ets.append(et)
                                if i2 >= 1:
                                    pv_pair(i2 - 1)
                            pv_pair(SK // 2 - 1)
                            # normalizer: chain-accumulate on VectorE (bf16 2x)
                            acc1 = acc_pool.tile([128, 2 * NC], BF, tag="acc1",
                                                 name=f"acc1_{h}_{jc}")
                            nc.vector.tensor_add(acc1[:], ets[0][:], ets[1][:])
                            for i2 in range(2, SK // 2):
                                nc.vector.tensor_add(acc1[:], acc1[:],
                                                     ets[i2][:])
                            acc2 = acc_pool.tile([128, NC], BF, tag="acc2",
                                                 name=f"acc2_{h}_{jc}")
                            nc.vector.tensor_add(acc2[:], acc1[:, 0:NC],
                                                 acc1[:, NC:2 * NC])
                            # single ones-matmul partition-sum of acc
                            rr = psr_pool.tile([128, NC], F32, tag="rr",
                                               name=f"rr{h}_{jc}")
                            nc.tensor.matmul(rr[:], ones_sk[:], acc2[:],
                                             start=True, stop=True)
                            rbs = rinv_pool.tile([128, NC], F32, tag="rbs",
                                                 name=f"rbs{h}_{jc}")
                            nc.vector.reciprocal_approx_fast(rbs[:], rr[:])
                            at = ats_pool.tile([Dh, NC], BF, tag="ats",
                                               name=f"ats{h}_{jc}")
                            nc.vector.tensor_mul(at[:], pv[:], rbs[:])
                            ats_t[(h, jc)] = at
                            # interleave o-proj of the previous chunk
                            if jc > 0:
                                for g in range(GPH * h, GPH * (h + 1)):
                                    emit_oproj_group(jc - 1, g)
                    for g in range(TPC * ND):
                        emit_oproj_group(JC - 1, g)

    nc.compile()
    return nc


def host_inputs(hidden_states, Wq, Wc, Wk, Wv, Wo, S=SEQ, Dh=HEAD_DIM,
                heads_per_core=HEADS_PER_CORE, n_cores=N_CORES):
    """Shard + preprocess full fp32 inputs into per-core bf16 in_maps."""
    D, L, H = D_MODEL, D_LATENT, heads_per_core
    KD = D // 128
    KQ = KD // Q4
    JA = S // NA
    scale = 1.0 / np.sqrt(Dh)
    pos = np.arange(S, dtype=np.float32)
    inv_freq = 1.0 / (ROPE_THETA ** (np.arange(0, Dh, 2, dtype=np.float32) / Dh))
    freqs = pos[:, None] * inv_freq
    emb = np.concatenate([freqs, freqs], axis=-1)      # [S, Dh]
    cosT = np.cos(emb).T.copy()                        # [Dh, S]
    sinT = np.sin(emb).T.copy()
    sinT[: Dh // 2] *= -1.0                            # sign baked for the swap
    cosq = cosT * scale
    sinq = sinT * scale
    cosk = cosT.astype(BF_NP)
    sink = sinT.astype(BF_NP)

    WcP = np.ascontiguousarray(
        Wc.astype(BF_NP).reshape(Q4, KQ, 128, L)
        .transpose(0, 2, 1, 3).reshape(Q4, 128, KQ * L))

    hw = heads_per_core * Dh
    in_maps = []
    hsQ_b = {}
    for b in range(BATCH):
        hsT = np.ascontiguousarray(hidden_states[b].T).astype(BF_NP)  # [D, S]
        # [JA, Q4, 128, KQ*NA] partition-major per (j, quarter) tile
        hsQ_b[b] = np.ascontiguousarray(
            hsT.reshape(Q4, KQ, 128, JA, NA)
            .transpose(3, 0, 2, 1, 4).reshape(JA, Q4, 128, KQ * NA))
    for c in range(n_cores):
        b, g = divmod(c, 4)
        cols = slice(g * hw, (g + 1) * hw)
        # permute query 512-blocks: core's latent shard (block g) first
        perm512 = [g] + [r for r in range(4) if r != g]
        perm_j = []
        for p in perm512:
            perm_j += [2 * p, 2 * p + 1]
        hsQ = np.ascontiguousarray(hsQ_b[b][perm_j])
        cq = np.ascontiguousarray(
            cosq.reshape(Dh, 4, 512)[:, perm512].reshape(Dh, S)).astype(BF_NP)
        sq = np.ascontiguousarray(
            sinq.reshape(Dh, 4, 512)[:, perm512].reshape(Dh, S)).astype(BF_NP)
        wq_c = np.ascontiguousarray(Wq[:, cols]).astype(BF_NP)   # [D, hw]
        WqP = np.ascontiguousarray(
            wq_c.reshape(KD, 128, H, Dh).transpose(2, 1, 0, 3)
            .reshape(H, 128, KD * Dh))
        in_maps.append({
            "hsQ": hsQ,
            "WqP": WqP,
            "WcP": WcP,
            "Wk": np.ascontiguousarray(Wk[:, cols]).astype(BF_NP),
            "Wv": np.ascontiguousarray(Wv[:, cols]).astype(BF_NP),
            "Wo": np.ascontiguousarray(Wo[cols, :]).astype(BF_NP),
            "cosq": cq, "sinq": sq, "cosk": cosk, "sink": sink,
        })
    return in_maps


_NC_CACHE = {}


def kernel(hidden_states, Wq, Wc, Wk, Wv, Wo):
    hidden_states = np.asarray(hidden_states, dtype=np.float32)
    if "nc" not in _NC_CACHE:
        _NC_CACHE["nc"] = build_nc()
    nc = _NC_CACHE["nc"]
    in_maps = host_inputs(hidden_states, np.asarray(Wq, np.float32),
                          np.asarray(Wc, np.float32), np.asarray(Wk, np.float32),
                          np.asarray(Wv, np.float32), np.asarray(Wo, np.float32))
    res = run_bass_kernel_spmd(nc, in_maps, list(range(N_CORES))).results
    B, S, D = BATCH, SEQ, D_MODEL
    out = np.zeros((B, S, D), dtype=np.float32)
    for c in range(N_CORES):
        b, g = divmod(c, 4)
        perm512 = [g] + [r for r in range(4) if r != g]
        o = res[c]["out"].astype(np.float32)          # [S, D] permuted rows
        for i, p in enumerate(perm512):
            out[b, p * 512:(p + 1) * 512] += o[i * 512:(i + 1) * 512]
    return out


# revision 22
# speedup vs baseline: 1.0050x; 1.0050x over previous
"""Multi-head latent attention (MLA) Bass kernel for 8 TRN2 NeuronCores.

Sharding: tensor-parallel over heads x data-parallel over batch.
Core c (0..7) owns batch b = c//4 and head group g = c%4 (8 heads of 32).
Each core computes, for its batch:
    latentT = (hs @ Wc)^T          (seq-sharded + AllGather within batch group)
    qT_h, kT_h (RoPE'd, transposed [head_dim, seq]) and v for its 8 heads
    attention with transposed scores [s_k, s_q]; the softmax normalizer is
    accumulated on VectorE (bf16 chunk adds) + ONE ones-matmul per tile
    o-proj fused into the attention loop: partial out -> [S, D] bf16
Host sums the 4 partials per batch in fp32.

Query 512-blocks are PERMUTED per core so the core's latent shard equals
query blocks 0-1: the latent projection reuses the q-proj activation tiles
(no separate hsl load). cos/sin tables are permuted to match; the host
un-permutes output rows.

DMA: all streamed tensors are packed partition-major on the host so each
DMA is one long contiguous run per partition (4-32KB packets, not 512B).
Queues: hs stream + out writes on qSync; weights on qScalar; latent
AllGather staging on the gpsimd SWDGE queue.
"""

import sys

for _p in ("/opt/trn_rl_repo", "/root/.axon_site/_ro/trn_rl_repo"):
    if _p not in sys.path:
        sys.path.insert(0, _p)

import numpy as np
import ml_dtypes

import concourse.bacc as bacc
import concourse.mybir as mybir
import concourse.tile as tile
from concourse import bass_isa
from concourse.bass_utils import run_bass_kernel_spmd

BF = mybir.dt.bfloat16
F32 = mybir.dt.float32
BF_NP = ml_dtypes.bfloat16

# Full-problem constants (hardcoded per the self-contained-kernel contract).
D_MODEL = 4096
D_LATENT = 512
NUM_HEADS = 32
HEAD_DIM = 128
ROPE_THETA = 10000.0
BATCH, SEQ = 2, 2048
N_CORES = 8
HEADS_PER_CORE = NUM_HEADS // 4  # 4 head groups x 2 batches = 8 cores
NA = 256                         # q-proj seq tile width
Q4 = 4                           # d_model quarters for hs/wc streaming


def build_nc(S=SEQ, D=D_MODEL, L=D_LATENT, H=HEADS_PER_CORE, Dh=HEAD_DIM,
             NC=512):
    """Build the single-core Bass program (SPMD across 8 cores)."""
    KD = D // 128     # contraction chunks over d_model
    KQ = KD // Q4     # chunks per quarter (8)
    LD = L // 128     # contraction chunks over d_latent
    JA = S // NA      # seq chunks in projection phase
    JC = S // NC      # seq chunks in attention phase
    SK = S // 128     # key-position chunks
    HD1 = H * Dh      # this core's total head width (1024)
    ND = D // NC      # output-column chunks
    SQ = S // 4       # this core's latent shard width (batch group of 4)
    NB = 512          # seq chunk width in phase B
    TPC = NC // 128   # seq tiles per attention chunk
    half = Dh // 2
    assert SK % 2 == 0 and NC == 512 and SQ == 2 * NA

    nc = bacc.Bacc("TRN2", target_bir_lowering=False)

    # partition-major packed streams (see host_inputs)
    hsq_d = nc.declare_dram_parameter("hsQ", [JA, Q4, 128, KQ * NA], BF,
                                      isOutput=False)
    wq_d = nc.declare_dram_parameter("WqP", [H, 128, KD * Dh], BF, isOutput=False)
    wc_d = nc.declare_dram_parameter("WcP", [Q4, 128, KQ * L], BF, isOutput=False)
    wk_d = nc.declare_dram_parameter("Wk", [L, HD1], BF, isOutput=False)
    wv_d = nc.declare_dram_parameter("Wv", [L, HD1], BF, isOutput=False)
    wo_d = nc.declare_dram_parameter("Wo", [HD1, D], BF, isOutput=False)
    cosq_d = nc.declare_dram_parameter("cosq", [Dh, S], BF, isOutput=False)
    sinq_d = nc.declare_dram_parameter("sinq", [Dh, S], BF, isOutput=False)
    cosk_d = nc.declare_dram_parameter("cosk", [Dh, S], BF, isOutput=False)
    sink_d = nc.declare_dram_parameter("sink", [Dh, S], BF, isOutput=False)
    out_d = nc.declare_dram_parameter("out", [S, D], BF, isOutput=True)
    latq_d = nc.dram_tensor("latq_dram", [L, SQ], BF)
    latg_d = nc.dram_tensor("latg_dram", [4 * L, SQ], BF)

    Exp = mybir.ActivationFunctionType.Exp

    with tile.TileContext(nc) as tc:
        with tc.tile_pool(name="consts", bufs=1) as const_pool, \
             tc.tile_pool(name="qT", bufs=1) as qT_pool:
            ones_sk = const_pool.tile([128, 128], BF)
            nc.vector.memset(ones_sk[:], 1.0)
            # preload the Exp activation table so the first real exp
            # (phase C) doesn't pay the ~2.7us table-load latency
            expwarm = const_pool.tile([128, 8], BF)
            nc.scalar.activation(expwarm[:], ones_sk[:, 0:8],
                                 mybir.ActivationFunctionType.Exp)
            qT_t = [qT_pool.tile([Dh, S], BF, name=f"qT{h}") for h in range(H)]

            # ========== Phase A: latent shard + AllGather; qT (RoPE) ==========
            with tc.tile_pool(name="wqA", bufs=1) as wqA_pool, \
                 tc.tile_pool(name="wcA", bufs=1) as wcA_pool, \
                 tc.tile_pool(name="hsq", bufs=3 * Q4) as hsq_pool, \
                 tc.tile_pool(name="ropeq", bufs=1) as ropeq_pool, \
                 tc.tile_pool(name="qraw", bufs=4) as qraw_pool, \
                 tc.tile_pool(name="tmpA", bufs=4) as tmpA_pool, \
                 tc.tile_pool(name="lq", bufs=4) as lq_pool, \
                 tc.tile_pool(name="psA", bufs=8, space="PSUM") as psA_pool:

                hs_tiles = {}

                # PE warm-up: dummy matmuls while the first DMAs stream in;
                # keeps the HAM clock-gate warm so real matmuls start at
                # full rate instead of 1.2 GHz
                warm_ps = psA_pool.tile([128, 128], F32, tag="psA",
                                        name="warmps")
                for _w in range(40):
                    nc.tensor.matmul(warm_ps[:], ones_sk[:], ones_sk[:],
                                     start=(_w == 0), stop=(_w == 39))

                def load_hs(j):
                    qs = []
                    for qd in range(Q4):
                        t = hsq_pool.tile([128, KQ * NA], BF, tag="hsq",
                                          name=f"hs_{j}_{qd}")
                        nc.sync.dma_start(out=t[:], in_=hsq_d[j, qd])
                        qs.append(t)
                    hs_tiles[j] = qs

                def hs_sl(j, kd):
                    return hs_tiles[j][kd // KQ][:, (kd % KQ) * NA:
                                                 (kd % KQ + 1) * NA]

                # startup: hs(j0,j1) quarters on the sync queue in latent
                # consumption order; wc on the scalar queue in parallel
                wc_t = [wcA_pool.tile([128, KQ * L], BF, name=f"wc{qd}")
                        for qd in range(Q4)]
                for j in (0, 1):
                    hs_tiles[j] = [hsq_pool.tile([128, KQ * NA], BF, tag="hsq",
                                                 name=f"hs_{j}_{qd}")
                                   for qd in range(Q4)]
                for qd in range(Q4):
                    nc.sync.dma_start(out=hs_tiles[0][qd][:], in_=hsq_d[0, qd])
                nc.scalar.dma_start(out=wc_t[0][:], in_=wc_d[0])
                nc.scalar.dma_start(out=wc_t[1][:], in_=wc_d[1])
                nc.sync.dma_start(out=wc_t[2][:], in_=wc_d[2])
                nc.sync.dma_start(out=wc_t[3][:], in_=wc_d[3])
                for qd in range(Q4):
                    nc.scalar.dma_start(out=hs_tiles[1][qd][:],
                                        in_=hsq_d[1, qd])

                def wc_sl(kd, ld):
                    return wc_t[kd // KQ][:, (kd % KQ) * L + ld * 128:
                                          (kd % KQ) * L + (ld + 1) * 128]

                # --- latent shard = query blocks 0-1 (per-core permuted) ---
                # quarter-major accumulation: consume each d_model quarter
                # fully before needing the next one's DMA
                ps_l = {}
                for jq in range(2):
                    for ld in range(LD):
                        ps_l[(jq, ld)] = psA_pool.tile(
                            [128, NA], F32, tag="psA", name=f"psLq{jq}_{ld}")
                for qd in range(Q4):
                    for jq in range(2):
                        for ld in range(LD):
                            for i in range(KQ):
                                kd = qd * KQ + i
                                nc.tensor.matmul(
                                    ps_l[(jq, ld)][:], wc_sl(kd, ld),
                                    hs_sl(jq, kd),
                                    start=(kd == 0), stop=(kd == KD - 1))
                for jq in range(2):
                    qq = slice(jq * NA, (jq + 1) * NA)
                    for ld in range(LD):
                        lq = lq_pool.tile([128, NA], BF, tag="lq",
                                          name=f"lq{jq}_{ld}")
                        nc.scalar.copy(lq[:], ps_l[(jq, ld)][:])
                        nc.gpsimd.dma_start(
                            out=latq_d[ld * 128:(ld + 1) * 128, qq],
                            in_=lq[:])
                nc.gpsimd.collective_compute(
                    "AllGather",
                    mybir.AluOpType.bypass,
                    replica_groups=[[0, 1, 2, 3], [4, 5, 6, 7]],
                    ins=[latq_d[:]],
                    outs=[latg_d[:]],
                )

                # --- qT loop; per-head wq so h=0 starts early ---
                cosq_sb = ropeq_pool.tile([Dh, S], BF)
                sinq_sb = ropeq_pool.tile([Dh, S], BF)
                wq_t = [wqA_pool.tile([128, KD * Dh], BF, name=f"wq{h}")
                        for h in range(H)]
                nc.scalar.dma_start(out=wq_t[0][:], in_=wq_d[0])
                nc.scalar.dma_start(out=cosq_sb[:], in_=cosq_d[:])
                nc.scalar.dma_start(out=sinq_sb[:], in_=sinq_d[:])
                for h in range(1, H):
                    nc.scalar.dma_start(out=wq_t[h][:], in_=wq_d[h])

                for j in range(JA):
                    if 2 <= j + 2 < JA:
                        load_hs(j + 2)
                    jj = slice(j * NA, (j + 1) * NA)
                    for h in range(H):
                        ps = psA_pool.tile([128, NA], F32, tag="psA",
                                           name=f"psQ{j}_{h}")
                        for kd in range(KD):
                            nc.tensor.matmul(
                                ps[:],
                                wq_t[h][:, kd * Dh:(kd + 1) * Dh],
                                hs_sl(j, kd),
                                start=(kd == 0), stop=(kd == KD - 1))
                        qr = qraw_pool.tile([128, NA], BF, tag="qraw",
                                            name=f"qr{j}_{h}")
                        nc.scalar.copy(qr[:], ps[:])
                        t2 = tmpA_pool.tile([128, NA], BF, tag="t2",
                                            name=f"t2q{j}_{h}")
                        t1 = tmpA_pool.tile([128, NA], BF, tag="t1",
                                            name=f"t1q{j}_{h}")
                        nc.vector.tensor_mul(t1[:], qr[:], cosq_sb[:, jj])
                        nc.vector.tensor_mul(t2[0:half, :], ps[half:Dh, :],
                                             sinq_sb[0:half, jj])
                        nc.vector.tensor_mul(t2[half:Dh, :], ps[0:half, :],
                                             sinq_sb[half:Dh, jj])
                        nc.vector.tensor_add(qT_t[h][:, jj], t1[:], t2[:])

            # ========== Phase B: kT (RoPE) + v;  Phase C+D fused ==========
            with tc.tile_pool(name="kT", bufs=1) as kT_pool, \
                 tc.tile_pool(name="v", bufs=1) as v_pool:
                kT_t = [kT_pool.tile([Dh, S], BF, name=f"kT{h}")
                        for h in range(H)]
                v_t = [v_pool.tile([128, HD1], BF, name=f"v{i}")
                       for i in range(SK)]

                with tc.tile_pool(name="latentT", bufs=1) as latent_pool, \
                     tc.tile_pool(name="wkv", bufs=1) as wkv_pool, \
                     tc.tile_pool(name="ropek", bufs=1) as ropek_pool, \
                     tc.tile_pool(name="kraw", bufs=4) as kraw_pool, \
                     tc.tile_pool(name="tmpB", bufs=6) as tmpB_pool, \
                     tc.tile_pool(name="psB", bufs=6, space="PSUM") as psB_pool:

                    latentT_t = [latent_pool.tile([128, S], BF, name=f"latT{ld}")
                                 for ld in range(LD)]
                    for ld in range(LD):
                        for r in range(4):
                            nc.gpsimd.dma_start(
                                out=latentT_t[ld][:, r * SQ:(r + 1) * SQ],
                                in_=latg_d[r * L + ld * 128:
                                           r * L + (ld + 1) * 128, :])
                    wk_t = [wkv_pool.tile([128, HD1], BF, name=f"wk{ld}")
                            for ld in range(LD)]
                    wv_t = [wkv_pool.tile([128, HD1], BF, name=f"wv{ld}")
                            for ld in range(LD)]
                    for ld in range(LD):
                        nc.sync.dma_start(
                            out=wk_t[ld][:], in_=wk_d[ld * 128:(ld + 1) * 128, :])
                        nc.sync.dma_start(
                            out=wv_t[ld][:], in_=wv_d[ld * 128:(ld + 1) * 128, :])
                    cosk_sb = ropek_pool.tile([Dh, S], BF)
                    sink_sb = ropek_pool.tile([Dh, S], BF)
                    nc.sync.dma_start(out=cosk_sb[:], in_=cosk_d[:])
                    nc.sync.dma_start(out=sink_sb[:], in_=sink_d[:])

                    # kT (h=0 first so attention starts early); v-expansion
                    # groups interleaved per head so the PE covers the DVE
                    # RoPE latency instead of stalling on psB reuse
                    NV = 512
                    for h in range(H):
                        for j in range(S // NB):
                            jj = slice(j * NB, (j + 1) * NB)
                            ps = psB_pool.tile([128, NB], F32, tag="psB",
                                               name=f"psK{h}_{j}")
                            for ld in range(LD):
                                nc.tensor.matmul(
                                    ps[:], wk_t[ld][:, h * Dh:(h + 1) * Dh],
                                    latentT_t[ld][:, jj],
                                    start=(ld == 0), stop=(ld == LD - 1))
                            kr = kraw_pool.tile([128, NB], BF, tag="kraw",
                                                name=f"kr{h}_{j}")
                            nc.scalar.copy(kr[:], ps[:])
                            t2 = tmpB_pool.tile([128, NB], BF, tag="t2b",
                                                name=f"t2k{h}_{j}")
                            t1 = tmpB_pool.tile([128, NB], BF, tag="t1b",
                                                name=f"t1k{h}_{j}")
                            nc.vector.tensor_mul(t1[:], kr[:], cosk_sb[:, jj])
                            nc.vector.tensor_mul(t2[0:half, :], ps[half:Dh, :],
                                                 sink_sb[0:half, jj])
                            nc.vector.tensor_mul(t2[half:Dh, :], ps[0:half, :],
                                                 sink_sb[half:Dh, jj])
                            nc.vector.tensor_add(kT_t[h][:, jj], t1[:], t2[:])
                        for k in range(4):
                            i = 2 * h + k // 2
                            cch = k % 2
                            cc = slice(cch * NV, (cch + 1) * NV)
                            ps = psB_pool.tile([128, NV], F32, tag="psB",
                                               name=f"psV{i}_{cch}")
                            for ld in range(LD):
                                nc.tensor.matmul(
                                    ps[:],
                                    latentT_t[ld][:, i * 128:(i + 1) * 128],
                                    wv_t[ld][:, cc],
                                    start=(ld == 0), stop=(ld == LD - 1))
                            nc.scalar.copy(v_t[i][:, cc], ps[:])

                # ===== Phase C+D fused: attention + output projection =====
                with tc.tile_pool(name="wo", bufs=1) as wo_pool, \
                     tc.tile_pool(name="ats", bufs=2 * H) as ats_pool, \
                     tc.tile_pool(name="ET", bufs=8) as et_pool, \
                     tc.tile_pool(name="accp", bufs=1) as acc_pool, \
                     tc.tile_pool(name="rinv", bufs=1) as rinv_pool, \
                     tc.tile_pool(name="outst", bufs=3) as outst_pool, \
                     tc.tile_pool(name="pssc", bufs=2, space="PSUM") as pssc_pool, \
                     tc.tile_pool(name="pspv", bufs=1, space="PSUM") as pspv_pool, \
                     tc.tile_pool(name="psr", bufs=1, space="PSUM") as psr_pool, \
                     tc.tile_pool(name="psD", bufs=2, space="PSUM") as psD_pool:

                    wo_t = [wo_pool.tile([128, D], BF, name=f"wo{h}")
                            for h in range(H)]
                    for h in range(H):
                        nc.sync.dma_start(out=wo_t[h][:],
                                          in_=wo_d[h * 128:(h + 1) * 128, :])

                    ats_t = {}   # (h, jc) -> [Dh, NC] bf16 tile
                    st_box = {}  # (jcp, tl) -> [128, 2*NC] staging tile

                    def emit_oproj_group(jcp, g):
                        """One o-proj PSUM group: local seq tile g//ND, out
                        cols g%ND, contracting all H heads. Output staged in
                        ncol pairs for 2KB-per-partition DMA runs."""
                        tl = g // ND
                        ncol = g % ND
                        t_abs = jcp * TPC + tl
                        tt = slice(t_abs * 128, (t_abs + 1) * 128)
                        ps = psD_pool.tile([128, NC], F32, tag="psD",
                                           name=f"psD{jcp}_{g}")
                        for h in range(H):
                            nc.tensor.matmul(
                                ps[:],
                                ats_t[(h, jcp)][:, tl * 128:(tl + 1) * 128],
                                wo_t[h][:, ncol * NC:(ncol + 1) * NC],
                                start=(h == 0), stop=(h == H - 1))
                        if ncol % 2 == 0:
                            st = outst_pool.tile([128, 2 * NC], BF, tag="outst",
                                                 name=f"outst{jcp}_{g}")
                            st_box[(jcp, tl)] = st
                            nc.vector.tensor_copy(st[:, 0:NC], ps[:])
                        else:
                            st = st_box[(jcp, tl)]
                            nc.vector.tensor_copy(st[:, NC:2 * NC], ps[:])
                            nc.sync.dma_start(
                                out=out_d[tt, (ncol - 1) * NC:(ncol + 1) * NC],
                                in_=st[:])

                    GPH = (TPC * ND) // H  # o-proj groups per h slot
                    for jc in range(JC):
                        jj = slice(jc * NC, (jc + 1) * NC)
                        for h in range(H):
                            # scores (transposed: keys on partitions) + exp;
                            # pv matmuls trail one chunk-pair behind so the
                            # PE stream never outruns ScalarE's exp rate
                            pv = pspv_pool.tile([Dh, NC], F32, tag="pv",
                                                name=f"pv{h}_{jc}")
                            ets = []

                            def pv_pair(i2p):
                                for i in (2 * i2p, 2 * i2p + 1):
                                    sl = ets[i // 2][:, (i % 2) * NC:
                                                     (i % 2 + 1) * NC]
                                    nc.tensor.matmul(
                                        pv[:], v_t[i][:, h * Dh:(h + 1) * Dh],
                                        sl, start=(i == 0),
                                        stop=(i == SK - 1))

                            for i2 in range(SK // 2):
                                ps2 = pssc_pool.tile([128, 2 * NC], F32,
                                                     tag="sc",
                                                     name=f"sc{h}_{jc}_{i2}")
                                for p in range(2):
                                    i = i2 * 2 + p
                                    nc.tensor.matmul(
                                        ps2[:, p * NC:(p + 1) * NC],
                                        kT_t[h][:, i * 128:(i + 1) * 128],
                                        qT_t[h][:, jj],
                                        start=True, stop=True)
                                et = et_pool.tile([128, 2 * NC], BF, tag="ET",
                                                  name=f"et{h}_{jc}_{i2}")
                                nc.scalar.activation(et[:], ps2[:], Exp)
                                ets.append(et)
                                if i2 >= 1:
                                    pv_pair(i2 - 1)
                            pv_pair(SK // 2 - 1)
                            # normalizer: chain-accumulate on VectorE (bf16 2x)
                            acc1 = acc_pool.tile([128, 2 * NC], BF, tag="acc1",
                                                 name=f"acc1_{h}_{jc}")
                            nc.vector.tensor_add(acc1[:], ets[0][:], ets[1][:])
                            for i2 in range(2, SK // 2):
                                nc.vector.tensor_add(acc1[:], acc1[:],
                                                     ets[i2][:])
                            acc2 = acc_pool.tile([128, NC], BF, tag="acc2",
                                                 name=f"acc2_{h}_{jc}")
                            nc.vector.tensor_add(acc2[:], acc1[:, 0:NC],
                                                 acc1[:, NC:2 * NC])
                            # single ones-matmul partition-sum of acc
                            rr = psr_pool.tile([128, NC], F32, tag="rr",
                                               name=f"rr{h}_{jc}")
                            nc.tensor.matmul(rr[:], ones_sk[:], acc2[:],
                                             start=True, stop=True)
                            rbs = rinv_pool.tile([128, NC], F32, tag="rbs",
                                                 name=f"rbs{h}_{jc}")
                            nc.vector.reciprocal_approx_fast(rbs[:], rr[:])
                            at = ats_pool.tile([Dh, NC], BF, tag="ats",
                                               name=f"ats{h}_{jc}")
                            nc.vector.tensor_mul(at[:], pv[:], rbs[:])
                            ats_t[(h, jc)] = at
                            # interleave o-proj of the previous chunk
                            if jc > 0:
                                for g in range(GPH * h, GPH * (h + 1)):
                                    emit_oproj_group(jc - 1, g)
                    for g in range(TPC * ND):
                        emit_oproj_group(JC - 1, g)

    nc.compile()
    return nc


def host_inputs(hidden_states, Wq, Wc, Wk, Wv, Wo, S=SEQ, Dh=HEAD_DIM,
                heads_per_core=HEADS_PER_CORE, n_cores=N_CORES):
    """Shard + preprocess full fp32 inputs into per-core bf16 in_maps."""
    D, L, H = D_MODEL, D_LATENT, heads_per_core
    KD = D // 128
    KQ = KD // Q4
    JA = S // NA
    scale = 1.0 / np.sqrt(Dh)
    pos = np.arange(S, dtype=np.float32)
    inv_freq = 1.0 / (ROPE_THETA ** (np.arange(0, Dh, 2, dtype=np.float32) / Dh))
    freqs = pos[:, None] * inv_freq
    emb = np.concatenate([freqs, freqs], axis=-1)      # [S, Dh]
    cosT = np.cos(emb).T.copy()                        # [Dh, S]
    sinT = np.sin(emb).T.copy()
    sinT[: Dh // 2] *= -1.0                            # sign baked for the swap
    cosq = cosT * scale
    sinq = sinT * scale
    cosk = cosT.astype(BF_NP)
    sink = sinT.astype(BF_NP)

    WcP = np.ascontiguousarray(
        Wc.astype(BF_NP).reshape(Q4, KQ, 128, L)
        .transpose(0, 2, 1, 3).reshape(Q4, 128, KQ * L))

    hw = heads_per_core * Dh
    in_maps = []
    hsQ_b = {}
    for b in range(BATCH):
        hsT = np.ascontiguousarray(hidden_states[b].T).astype(BF_NP)  # [D, S]
        # [JA, Q4, 128, KQ*NA] partition-major per (j, quarter) tile
        hsQ_b[b] = np.ascontiguousarray(
            hsT.reshape(Q4, KQ, 128, JA, NA)
            .transpose(3, 0, 2, 1, 4).reshape(JA, Q4, 128, KQ * NA))
    for c in range(n_cores):
        b, g = divmod(c, 4)
        cols = slice(g * hw, (g + 1) * hw)
        # permute query 512-blocks: core's latent shard (block g) first
        perm512 = [g] + [r for r in range(4) if r != g]
        perm_j = []
        for p in perm512:
            perm_j += [2 * p, 2 * p + 1]
        hsQ = np.ascontiguousarray(hsQ_b[b][perm_j])
        cq = np.ascontiguousarray(
            cosq.reshape(Dh, 4, 512)[:, perm512].reshape(Dh, S)).astype(BF_NP)
        sq = np.ascontiguousarray(
            sinq.reshape(Dh, 4, 512)[:, perm512].reshape(Dh, S)).astype(BF_NP)
        wq_c = np.ascontiguousarray(Wq[:, cols]).astype(BF_NP)   # [D, hw]
        WqP = np.ascontiguousarray(
            wq_c.reshape(KD, 128, H, Dh).transpose(2, 1, 0, 3)
            .reshape(H, 128, KD * Dh))
        in_maps.append({
            "hsQ": hsQ,
            "WqP": WqP,
            "WcP": WcP,
            "Wk": np.ascontiguousarray(Wk[:, cols]).astype(BF_NP),
            "Wv": np.ascontiguousarray(Wv[:, cols]).astype(BF_NP),
            "Wo": np.ascontiguousarray(Wo[cols, :]).astype(BF_NP),
            "cosq": cq, "sinq": sq, "cosk": cosk, "sink": sink,
        })
    return in_maps


_NC_CACHE = {}


def kernel(hidden_states, Wq, Wc, Wk, Wv, Wo):
    hidden_states = np.asarray(hidden_states, dtype=np.float32)
    if "nc" not in _NC_CACHE:
        _NC_CACHE["nc"] = build_nc()
    nc = _NC_CACHE["nc"]
    in_maps = host_inputs(hidden_states, np.asarray(Wq, np.float32),
                          np.asarray(Wc, np.float32), np.asarray(Wk, np.float32),
                          np.asarray(Wv, np.float32), np.asarray(Wo, np.float32))
    res = run_bass_kernel_spmd(nc, in_maps, list(range(N_CORES))).results
    B, S, D = BATCH, SEQ, D_MODEL
    out = np.zeros((B, S, D), dtype=np.float32)
    for c in range(N_CORES):
        b, g = divmod(c, 4)
        perm512 = [g] + [r for r in range(4) if r != g]
        o = res[c]["out"].astype(np.float32)          # [S, D] permuted rows
        for i, p in enumerate(perm512):
            out[b, p * 512:(p + 1) * 512] += o[i * 512:(i + 1) * 512]
    return out


# revision 23
# speedup vs baseline: 1.0137x; 1.0087x over previous
"""Multi-head latent attention (MLA) Bass kernel for 8 TRN2 NeuronCores.

Sharding: tensor-parallel over heads x data-parallel over batch.
Core c (0..7) owns batch b = c//4 and head group g = c%4 (8 heads of 32).
Each core computes, for its batch:
    latentT = (hs @ Wc)^T          (seq-sharded + AllGather within batch group)
    qT_h, kT_h (RoPE'd, transposed [head_dim, seq]) and v for its 8 heads
    attention with transposed scores [s_k, s_q]; the softmax normalizer is
    accumulated on VectorE (bf16 chunk adds) + ONE ones-matmul per tile
    o-proj fused into the attention loop: partial out -> [S, D] bf16
Host sums the 4 partials per batch in fp32.

Query 512-blocks are PERMUTED per core so the core's latent shard equals
query blocks 0-1: the latent projection reuses the q-proj activation tiles
(no separate hsl load). cos/sin tables are permuted to match; the host
un-permutes output rows.

DMA: all streamed tensors are packed partition-major on the host so each
DMA is one long contiguous run per partition (4-32KB packets, not 512B).
Queues: hs stream + out writes on qSync; weights on qScalar; latent
AllGather staging on the gpsimd SWDGE queue.
"""

import sys

for _p in ("/opt/trn_rl_repo", "/root/.axon_site/_ro/trn_rl_repo"):
    if _p not in sys.path:
        sys.path.insert(0, _p)

import numpy as np
import ml_dtypes

import concourse.bacc as bacc
import concourse.mybir as mybir
import concourse.tile as tile
from concourse import bass_isa
from concourse.bass_utils import run_bass_kernel_spmd

BF = mybir.dt.bfloat16
F32 = mybir.dt.float32
BF_NP = ml_dtypes.bfloat16

# Full-problem constants (hardcoded per the self-contained-kernel contract).
D_MODEL = 4096
D_LATENT = 512
NUM_HEADS = 32
HEAD_DIM = 128
ROPE_THETA = 10000.0
BATCH, SEQ = 2, 2048
N_CORES = 8
HEADS_PER_CORE = NUM_HEADS // 4  # 4 head groups x 2 batches = 8 cores
NA = 256                         # q-proj seq tile width
Q4 = 4                           # d_model quarters for hs/wc streaming


def build_nc(S=SEQ, D=D_MODEL, L=D_LATENT, H=HEADS_PER_CORE, Dh=HEAD_DIM,
             NC=512):
    """Build the single-core Bass program (SPMD across 8 cores)."""
    KD = D // 128     # contraction chunks over d_model
    KQ = KD // Q4     # chunks per quarter (8)
    LD = L // 128     # contraction chunks over d_latent
    JA = S // NA      # seq chunks in projection phase
    JC = S // NC      # seq chunks in attention phase
    SK = S // 128     # key-position chunks
    HD1 = H * Dh      # this core's total head width (1024)
    ND = D // NC      # output-column chunks
    SQ = S // 4       # this core's latent shard width (batch group of 4)
    NB = 512          # seq chunk width in phase B
    TPC = NC // 128   # seq tiles per attention chunk
    half = Dh // 2
    assert SK % 2 == 0 and NC == 512 and SQ == 2 * NA

    nc = bacc.Bacc("TRN2", target_bir_lowering=False)

    # partition-major packed streams (see host_inputs)
    hsq_d = nc.declare_dram_parameter("hsQ", [JA, Q4, 128, KQ * NA], BF,
                                      isOutput=False)
    wq_d = nc.declare_dram_parameter("WqP", [H, 128, KD * Dh], BF, isOutput=False)
    wc_d = nc.declare_dram_parameter("WcP", [Q4, 128, KQ * L], BF, isOutput=False)
    wk_d = nc.declare_dram_parameter("Wk", [L, HD1], BF, isOutput=False)
    wv_d = nc.declare_dram_parameter("Wv", [L, HD1], BF, isOutput=False)
    wo_d = nc.declare_dram_parameter("Wo", [HD1, D], BF, isOutput=False)
    cosq_d = nc.declare_dram_parameter("cosq", [Dh, S], BF, isOutput=False)
    sinq_d = nc.declare_dram_parameter("sinq", [Dh, S], BF, isOutput=False)
    cosk_d = nc.declare_dram_parameter("cosk", [Dh, S], BF, isOutput=False)
    sink_d = nc.declare_dram_parameter("sink", [Dh, S], BF, isOutput=False)
    out_d = nc.declare_dram_parameter("out", [S, D], BF, isOutput=True)
    latq_d = nc.dram_tensor("latq_dram", [L, SQ], BF)
    latg_d = nc.dram_tensor("latg_dram", [4 * L, SQ], BF)

    Exp = mybir.ActivationFunctionType.Exp

    with tile.TileContext(nc) as tc:
        with tc.tile_pool(name="consts", bufs=1) as const_pool, \
             tc.tile_pool(name="qT", bufs=1) as qT_pool:
            ones_sk = const_pool.tile([128, 128], BF)
            nc.vector.memset(ones_sk[:], 1.0)
            # preload the Exp activation table so the first real exp
            # (phase C) doesn't pay the ~2.7us table-load latency
            expwarm = const_pool.tile([128, 8], BF)
            nc.scalar.activation(expwarm[:], ones_sk[:, 0:8],
                                 mybir.ActivationFunctionType.Exp)
            qT_t = [qT_pool.tile([Dh, S], BF, name=f"qT{h}") for h in range(H)]

            # ========== Phase A: latent shard + AllGather; qT (RoPE) ==========
            with tc.tile_pool(name="wqA", bufs=1) as wqA_pool, \
                 tc.tile_pool(name="wcA", bufs=1) as wcA_pool, \
                 tc.tile_pool(name="hsq", bufs=3 * Q4) as hsq_pool, \
                 tc.tile_pool(name="ropeq", bufs=1) as ropeq_pool, \
                 tc.tile_pool(name="qraw", bufs=4) as qraw_pool, \
                 tc.tile_pool(name="tmpA", bufs=4) as tmpA_pool, \
                 tc.tile_pool(name="lq", bufs=4) as lq_pool, \
                 tc.tile_pool(name="psA", bufs=8, space="PSUM") as psA_pool:

                hs_tiles = {}

                # PE warm-up: dummy matmuls while the first DMAs stream in;
                # keeps the HAM clock-gate warm so real matmuls start at
                # full rate instead of 1.2 GHz
                warm_ps = psA_pool.tile([128, 128], F32, tag="psA",
                                        name="warmps")
                for _w in range(56):
                    nc.tensor.matmul(warm_ps[:], ones_sk[:], ones_sk[:],
                                     start=(_w == 0), stop=(_w == 55))

                def load_hs(j):
                    qs = []
                    for qd in range(Q4):
                        t = hsq_pool.tile([128, KQ * NA], BF, tag="hsq",
                                          name=f"hs_{j}_{qd}")
                        nc.sync.dma_start(out=t[:], in_=hsq_d[j, qd])
                        qs.append(t)
                    hs_tiles[j] = qs

                def hs_sl(j, kd):
                    return hs_tiles[j][kd // KQ][:, (kd % KQ) * NA:
                                                 (kd % KQ + 1) * NA]

                # startup: hs(j0,j1) quarters on the sync queue in latent
                # consumption order; wc on the scalar queue in parallel
                wc_t = [wcA_pool.tile([128, KQ * L], BF, name=f"wc{qd}")
                        for qd in range(Q4)]
                for j in (0, 1):
                    hs_tiles[j] = [hsq_pool.tile([128, KQ * NA], BF, tag="hsq",
                                                 name=f"hs_{j}_{qd}")
                                   for qd in range(Q4)]
                for qd in range(Q4):
                    nc.sync.dma_start(out=hs_tiles[0][qd][:], in_=hsq_d[0, qd])
                nc.scalar.dma_start(out=wc_t[0][:], in_=wc_d[0])
                nc.scalar.dma_start(out=wc_t[1][:], in_=wc_d[1])
                nc.sync.dma_start(out=wc_t[2][:], in_=wc_d[2])
                nc.sync.dma_start(out=wc_t[3][:], in_=wc_d[3])
                for qd in range(Q4):
                    nc.scalar.dma_start(out=hs_tiles[1][qd][:],
                                        in_=hsq_d[1, qd])

                def wc_sl(kd, ld):
                    return wc_t[kd // KQ][:, (kd % KQ) * L + ld * 128:
                                          (kd % KQ) * L + (ld + 1) * 128]

                # --- latent shard = query blocks 0-1 (per-core permuted) ---
                # quarter-major accumulation: consume each d_model quarter
                # fully before needing the next one's DMA
                ps_l = {}
                for jq in range(2):
                    for ld in range(LD):
                        ps_l[(jq, ld)] = psA_pool.tile(
                            [128, NA], F32, tag="psA", name=f"psLq{jq}_{ld}")
                for qd in range(Q4):
                    for jq in range(2):
                        for ld in range(LD):
                            for i in range(KQ):
                                kd = qd * KQ + i
                                nc.tensor.matmul(
                                    ps_l[(jq, ld)][:], wc_sl(kd, ld),
                                    hs_sl(jq, kd),
                                    start=(kd == 0), stop=(kd == KD - 1))
                for jq in range(2):
                    qq = slice(jq * NA, (jq + 1) * NA)
                    for ld in range(LD):
                        lq = lq_pool.tile([128, NA], BF, tag="lq",
                                          name=f"lq{jq}_{ld}")
                        nc.scalar.copy(lq[:], ps_l[(jq, ld)][:])
                        nc.gpsimd.dma_start(
                            out=latq_d[ld * 128:(ld + 1) * 128, qq],
                            in_=lq[:])
                nc.gpsimd.collective_compute(
                    "AllGather",
                    mybir.AluOpType.bypass,
                    replica_groups=[[0, 1, 2, 3], [4, 5, 6, 7]],
                    ins=[latq_d[:]],
                    outs=[latg_d[:]],
                )

                # --- qT loop; per-head wq so h=0 starts early ---
                cosq_sb = ropeq_pool.tile([Dh, S], BF)
                sinq_sb = ropeq_pool.tile([Dh, S], BF)
                wq_t = [wqA_pool.tile([128, KD * Dh], BF, name=f"wq{h}")
                        for h in range(H)]
                nc.scalar.dma_start(out=wq_t[0][:], in_=wq_d[0])
                nc.scalar.dma_start(out=cosq_sb[:], in_=cosq_d[:])
                nc.scalar.dma_start(out=sinq_sb[:], in_=sinq_d[:])
                for h in range(1, H):
                    nc.scalar.dma_start(out=wq_t[h][:], in_=wq_d[h])

                for j in range(JA):
                    if 2 <= j + 2 < JA:
                        load_hs(j + 2)
                    jj = slice(j * NA, (j + 1) * NA)
                    for h in range(H):
                        ps = psA_pool.tile([128, NA], F32, tag="psA",
                                           name=f"psQ{j}_{h}")
                        for kd in range(KD):
                            nc.tensor.matmul(
                                ps[:],
                                wq_t[h][:, kd * Dh:(kd + 1) * Dh],
                                hs_sl(j, kd),
                                start=(kd == 0), stop=(kd == KD - 1))
                        qr = qraw_pool.tile([128, NA], BF, tag="qraw",
                                            name=f"qr{j}_{h}")
                        nc.scalar.copy(qr[:], ps[:])
                        t2 = tmpA_pool.tile([128, NA], BF, tag="t2",
                                            name=f"t2q{j}_{h}")
                        t1 = tmpA_pool.tile([128, NA], BF, tag="t1",
                                            name=f"t1q{j}_{h}")
                        nc.vector.tensor_mul(t1[:], qr[:], cosq_sb[:, jj])
                        nc.vector.tensor_mul(t2[0:half, :], ps[half:Dh, :],
                                             sinq_sb[0:half, jj])
                        nc.vector.tensor_mul(t2[half:Dh, :], ps[0:half, :],
                                             sinq_sb[half:Dh, jj])
                        nc.vector.tensor_add(qT_t[h][:, jj], t1[:], t2[:])

            # ========== Phase B: kT (RoPE) + v;  Phase C+D fused ==========
            with tc.tile_pool(name="kT", bufs=1) as kT_pool, \
                 tc.tile_pool(name="v", bufs=1) as v_pool:
                kT_t = [kT_pool.tile([Dh, S], BF, name=f"kT{h}")
                        for h in range(H)]
                v_t = [v_pool.tile([128, HD1], BF, name=f"v{i}")
                       for i in range(SK)]

                with tc.tile_pool(name="latentT", bufs=1) as latent_pool, \
                     tc.tile_pool(name="wkv", bufs=1) as wkv_pool, \
                     tc.tile_pool(name="ropek", bufs=1) as ropek_pool, \
                     tc.tile_pool(name="kraw", bufs=4) as kraw_pool, \
                     tc.tile_pool(name="tmpB", bufs=6) as tmpB_pool, \
                     tc.tile_pool(name="psB", bufs=6, space="PSUM") as psB_pool:

                    latentT_t = [latent_pool.tile([128, S], BF, name=f"latT{ld}")
                                 for ld in range(LD)]
                    for ld in range(LD):
                        for r in range(4):
                            nc.gpsimd.dma_start(
                                out=latentT_t[ld][:, r * SQ:(r + 1) * SQ],
                                in_=latg_d[r * L + ld * 128:
                                           r * L + (ld + 1) * 128, :])
                    wk_t = [wkv_pool.tile([128, HD1], BF, name=f"wk{ld}")
                            for ld in range(LD)]
                    wv_t = [wkv_pool.tile([128, HD1], BF, name=f"wv{ld}")
                            for ld in range(LD)]
                    for ld in range(LD):
                        nc.sync.dma_start(
                            out=wk_t[ld][:], in_=wk_d[ld * 128:(ld + 1) * 128, :])
                        nc.sync.dma_start(
                            out=wv_t[ld][:], in_=wv_d[ld * 128:(ld + 1) * 128, :])
                    cosk_sb = ropek_pool.tile([Dh, S], BF)
                    sink_sb = ropek_pool.tile([Dh, S], BF)
                    nc.sync.dma_start(out=cosk_sb[:], in_=cosk_d[:])
                    nc.sync.dma_start(out=sink_sb[:], in_=sink_d[:])

                    # kT (h=0 first so attention starts early); v-expansion
                    # groups interleaved per head so the PE covers the DVE
                    # RoPE latency instead of stalling on psB reuse
                    NV = 512
                    for h in range(H):
                        for j in range(S // NB):
                            jj = slice(j * NB, (j + 1) * NB)
                            ps = psB_pool.tile([128, NB], F32, tag="psB",
                                               name=f"psK{h}_{j}")
                            for ld in range(LD):
                                nc.tensor.matmul(
                                    ps[:], wk_t[ld][:, h * Dh:(h + 1) * Dh],
                                    latentT_t[ld][:, jj],
                                    start=(ld == 0), stop=(ld == LD - 1))
                            kr = kraw_pool.tile([128, NB], BF, tag="kraw",
                                                name=f"kr{h}_{j}")
                            nc.scalar.copy(kr[:], ps[:])
                            t2 = tmpB_pool.tile([128, NB], BF, tag="t2b",
                                                name=f"t2k{h}_{j}")
                            t1 = tmpB_pool.tile([128, NB], BF, tag="t1b",
                                                name=f"t1k{h}_{j}")
                            nc.vector.tensor_mul(t1[:], kr[:], cosk_sb[:, jj])
                            nc.vector.tensor_mul(t2[0:half, :], ps[half:Dh, :],
                                                 sink_sb[0:half, jj])
                            nc.vector.tensor_mul(t2[half:Dh, :], ps[0:half, :],
                                                 sink_sb[half:Dh, jj])
                            nc.vector.tensor_add(kT_t[h][:, jj], t1[:], t2[:])
                        for k in range(4):
                            i = 2 * h + k // 2
                            cch = k % 2
                            cc = slice(cch * NV, (cch + 1) * NV)
                            ps = psB_pool.tile([128, NV], F32, tag="psB",
                                               name=f"psV{i}_{cch}")
                            for ld in range(LD):
                                nc.tensor.matmul(
                                    ps[:],
                                    latentT_t[ld][:, i * 128:(i + 1) * 128],
                                    wv_t[ld][:, cc],
                                    start=(ld == 0), stop=(ld == LD - 1))
                            nc.scalar.copy(v_t[i][:, cc], ps[:])

                # ===== Phase C+D fused: attention + output projection =====
                with tc.tile_pool(name="wo", bufs=1) as wo_pool, \
                     tc.tile_pool(name="ats", bufs=2 * H) as ats_pool, \
                     tc.tile_pool(name="ET", bufs=8) as et_pool, \
                     tc.tile_pool(name="accp", bufs=1) as acc_pool, \
                     tc.tile_pool(name="rinv", bufs=1) as rinv_pool, \
                     tc.tile_pool(name="outst", bufs=3) as outst_pool, \
                     tc.tile_pool(name="psD", bufs=2, space="PSUM") as psD_pool, \
                     tc.tile_pool(name="psr", bufs=1, space="PSUM") as psr_pool, \
                     tc.tile_pool(name="pspv", bufs=1, space="PSUM") as pspv_pool, \
                     tc.tile_pool(name="pssc", bufs=2, space="PSUM") as pssc_pool:

                    wo_t = [wo_pool.tile([128, D], BF, name=f"wo{h}")
                            for h in range(H)]
                    for h in range(H):
                        nc.sync.dma_start(out=wo_t[h][:],
                                          in_=wo_d[h * 128:(h + 1) * 128, :])

                    ats_t = {}   # (h, jc) -> [Dh, NC] bf16 tile
                    st_box = {}  # (jcp, tl) -> [128, 2*NC] staging tile

                    def emit_oproj_group(jcp, g):
                        """One o-proj PSUM group: local seq tile g//ND, out
                        cols g%ND, contracting all H heads. Output staged in
                        ncol pairs for 2KB-per-partition DMA runs."""
                        tl = g // ND
                        ncol = g % ND
                        t_abs = jcp * TPC + tl
                        tt = slice(t_abs * 128, (t_abs + 1) * 128)
                        ps = psD_pool.tile([128, NC], F32, tag="psD",
                                           name=f"psD{jcp}_{g}")
                        for h in range(H):
                            nc.tensor.matmul(
                                ps[:],
                                ats_t[(h, jcp)][:, tl * 128:(tl + 1) * 128],
                                wo_t[h][:, ncol * NC:(ncol + 1) * NC],
                                start=(h == 0), stop=(h == H - 1))
                        if ncol % 2 == 0:
                            st = outst_pool.tile([128, 2 * NC], BF, tag="outst",
                                                 name=f"outst{jcp}_{g}")
                            st_box[(jcp, tl)] = st
                            nc.vector.tensor_copy(st[:, 0:NC], ps[:])
                        else:
                            st = st_box[(jcp, tl)]
                            nc.vector.tensor_copy(st[:, NC:2 * NC], ps[:])
                            nc.sync.dma_start(
                                out=out_d[tt, (ncol - 1) * NC:(ncol + 1) * NC],
                                in_=st[:])

                    GPH = (TPC * ND) // H  # o-proj groups per h slot
                    for jc in range(JC):
                        jj = slice(jc * NC, (jc + 1) * NC)
                        for h in range(H):
                            # scores (transposed: keys on partitions) + exp;
                            # pv matmuls trail one chunk-pair behind so the
                            # PE stream never outruns ScalarE's exp rate
                            pv = pspv_pool.tile([Dh, NC], F32, tag="pv",
                                                name=f"pv{h}_{jc}")
                            ets = []

                            def pv_pair(i2p):
                                for i in (2 * i2p, 2 * i2p + 1):
                                    sl = ets[i // 2][:, (i % 2) * NC:
                                                     (i % 2 + 1) * NC]
                                    nc.tensor.matmul(
                                        pv[:], v_t[i][:, h * Dh:(h + 1) * Dh],
                                        sl, start=(i == 0),
                                        stop=(i == SK - 1))

                            for i2 in range(SK // 2):
                                ps2 = pssc_pool.tile([128, 2 * NC], F32,
                                                     tag="sc",
                                                     name=f"sc{h}_{jc}_{i2}")
                                for p in range(2):
                                    i = i2 * 2 + p
                                    nc.tensor.matmul(
                                        ps2[:, p * NC:(p + 1) * NC],
                                        kT_t[h][:, i * 128:(i + 1) * 128],
                                        qT_t[h][:, jj],
                                        start=True, stop=True)
                                et = et_pool.tile([128, 2 * NC], BF, tag="ET",
                                                  name=f"et{h}_{jc}_{i2}")
                                nc.scalar.activation(et[:], ps2[:], Exp)
                                ets.append(et)
                                if i2 >= 1:
                                    pv_pair(i2 - 1)
                            pv_pair(SK // 2 - 1)
                            # normalizer: chain-accumulate on VectorE (bf16 2x)
                            acc1 = acc_pool.tile([128, 2 * NC], BF, tag="acc1",
                                                 name=f"acc1_{h}_{jc}")
                            nc.vector.tensor_add(acc1[:], ets[0][:], ets[1][:])
                            for i2 in range(2, SK // 2):
                                nc.vector.tensor_add(acc1[:], acc1[:],
                                                     ets[i2][:])
                            acc2 = acc_pool.tile([128, NC], BF, tag="acc2",
                                                 name=f"acc2_{h}_{jc}")
                            nc.vector.tensor_add(acc2[:], acc1[:, 0:NC],
                                                 acc1[:, NC:2 * NC])
                            # single ones-matmul partition-sum of acc
                            rr = psr_pool.tile([128, NC], F32, tag="rr",
                                               name=f"rr{h}_{jc}")
                            nc.tensor.matmul(rr[:], ones_sk[:], acc2[:],
                                             start=True, stop=True)
                            rbs = rinv_pool.tile([128, NC], F32, tag="rbs",
                                                 name=f"rbs{h}_{jc}")
                            nc.vector.reciprocal_approx_fast(rbs[:], rr[:])
                            at = ats_pool.tile([Dh, NC], BF, tag="ats",
                                               name=f"ats{h}_{jc}")
                            nc.vector.tensor_mul(at[:], pv[:], rbs[:])
                            ats_t[(h, jc)] = at
                            # interleave o-proj of the previous chunk
                            if jc > 0:
                                for g in range(GPH * h, GPH * (h + 1)):
                                    emit_oproj_group(jc - 1, g)
                    for g in range(TPC * ND):
                        emit_oproj_group(JC - 1, g)

    nc.compile()
    return nc


def host_inputs(hidden_states, Wq, Wc, Wk, Wv, Wo, S=SEQ, Dh=HEAD_DIM,
                heads_per_core=HEADS_PER_CORE, n_cores=N_CORES):
    """Shard + preprocess full fp32 inputs into per-core bf16 in_maps."""
    D, L, H = D_MODEL, D_LATENT, heads_per_core
    KD = D // 128
    KQ = KD // Q4
    JA = S // NA
    scale = 1.0 / np.sqrt(Dh)
    pos = np.arange(S, dtype=np.float32)
    inv_freq = 1.0 / (ROPE_THETA ** (np.arange(0, Dh, 2, dtype=np.float32) / Dh))
    freqs = pos[:, None] * inv_freq
    emb = np.concatenate([freqs, freqs], axis=-1)      # [S, Dh]
    cosT = np.cos(emb).T.copy()                        # [Dh, S]
    sinT = np.sin(emb).T.copy()
    sinT[: Dh // 2] *= -1.0                            # sign baked for the swap
    cosq = cosT * scale
    sinq = sinT * scale
    cosk = cosT.astype(BF_NP)
    sink = sinT.astype(BF_NP)

    WcP = np.ascontiguousarray(
        Wc.astype(BF_NP).reshape(Q4, KQ, 128, L)
        .transpose(0, 2, 1, 3).reshape(Q4, 128, KQ * L))

    hw = heads_per_core * Dh
    in_maps = []
    hsQ_b = {}
    for b in range(BATCH):
        hsT = np.ascontiguousarray(hidden_states[b].T).astype(BF_NP)  # [D, S]
        # [JA, Q4, 128, KQ*NA] partition-major per (j, quarter) tile
        hsQ_b[b] = np.ascontiguousarray(
            hsT.reshape(Q4, KQ, 128, JA, NA)
            .transpose(3, 0, 2, 1, 4).reshape(JA, Q4, 128, KQ * NA))
    for c in range(n_cores):
        b, g = divmod(c, 4)
        cols = slice(g * hw, (g + 1) * hw)
        # permute query 512-blocks: core's latent shard (block g) first
        perm512 = [g] + [r for r in range(4) if r != g]
        perm_j = []
        for p in perm512:
            perm_j += [2 * p, 2 * p + 1]
        hsQ = np.ascontiguousarray(hsQ_b[b][perm_j])
        cq = np.ascontiguousarray(
            cosq.reshape(Dh, 4, 512)[:, perm512].reshape(Dh, S)).astype(BF_NP)
        sq = np.ascontiguousarray(
            sinq.reshape(Dh, 4, 512)[:, perm512].reshape(Dh, S)).astype(BF_NP)
        wq_c = np.ascontiguousarray(Wq[:, cols]).astype(BF_NP)   # [D, hw]
        WqP = np.ascontiguousarray(
            wq_c.reshape(KD, 128, H, Dh).transpose(2, 1, 0, 3)
            .reshape(H, 128, KD * Dh))
        in_maps.append({
            "hsQ": hsQ,
            "WqP": WqP,
            "WcP": WcP,
            "Wk": np.ascontiguousarray(Wk[:, cols]).astype(BF_NP),
            "Wv": np.ascontiguousarray(Wv[:, cols]).astype(BF_NP),
            "Wo": np.ascontiguousarray(Wo[cols, :]).astype(BF_NP),
            "cosq": cq, "sinq": sq, "cosk": cosk, "sink": sink,
        })
    return in_maps


_NC_CACHE = {}


def kernel(hidden_states, Wq, Wc, Wk, Wv, Wo):
    hidden_states = np.asarray(hidden_states, dtype=np.float32)
    if "nc" not in _NC_CACHE:
        _NC_CACHE["nc"] = build_nc()
    nc = _NC_CACHE["nc"]
    in_maps = host_inputs(hidden_states, np.asarray(Wq, np.float32),
                          np.asarray(Wc, np.float32), np.asarray(Wk, np.float32),
                          np.asarray(Wv, np.float32), np.asarray(Wo, np.float32))
    res = run_bass_kernel_spmd(nc, in_maps, list(range(N_CORES))).results
    B, S, D = BATCH, SEQ, D_MODEL
    out = np.zeros((B, S, D), dtype=np.float32)
    for c in range(N_CORES):
        b, g = divmod(c, 4)
        perm512 = [g] + [r for r in range(4) if r != g]
        o = res[c]["out"].astype(np.float32)          # [S, D] permuted rows
        for i, p in enumerate(perm512):
            out[b, p * 512:(p + 1) * 512] += o[i * 512:(i + 1) * 512]
    return out
